# revision 30
# baseline (speedup 1.0000x reference)
"""Trainium2 Bass kernel for ConvTranspose3d(3->16,k3,s2,p1) + BatchNorm3d(train) + 2x AvgPool3d(2).

Algorithm (per core, batch-sharded 4 samples/core over 8 cores):
  - ConvT decomposes into 8 "phases" (output parity per spatial dim). With the input
    replicated 8x as flat-shifted rows V[(cin,dd,dh,dw)] = x[cin].flat[shift:], one
    matmul per output-base-position chunk computes all 8 phases x 16 channels at once
    (lhsT = phase-weight matrix [24,128], rhs = V rows [24,N], out = [128,N] PSUM).
  - BN statistics (sum, sumsq per channel) come from bn_stats over each PSUM chunk,
    region-split (interior/faces/edges/corner) so invalid phase outputs are excluded,
    then a cross-core AllReduce (sync-BN).
  - The two AvgPools collapse into a 4x4x4 block-sum of conv output positions 0..59,
    which equals a stride-2 3x3x3 conv of x with a host-precomputed effective kernel
    (BN is per-channel affine, so pooling commutes with it). Computed by a second,
    tiny 8-pass matmul stack; normalization is applied as a single fused affine.
"""

import numpy as np

S = 32768          # 32*32*32 flat spatial
SPC = 4            # samples per core
NCORES = 8
PAD = 2048
XCAT = SPC * 3 * S + PAD


# ---------------------------------------------------------------------------
# host-side constant construction (weight transforms etc.)
# ---------------------------------------------------------------------------
def _host_consts(weight, gamma, beta):
    w = np.asarray(weight, np.float32)            # (3,16,3,3,3)

    # W128[(dd,dh,dw,cin), 16*P + c], P = 4*ed+2*eh+ew   (delta-major rows)
    W128 = np.zeros((24, 128), np.float32)
    for cin in range(3):
        for dd in range(2):
            for dh in range(2):
                for dw in range(2):
                    k = (dd * 4 + dh * 2 + dw) * 3 + cin
                    for P in range(8):
                        ed, eh, ew = P >> 2 & 1, P >> 1 & 1, P & 1
                        ok, ts = True, []
                        for e, d in ((ed, dd), (eh, dh), (ew, dw)):
                            if e == 0:
                                if d != 0:
                                    ok = False
                                    break
                                ts.append(1)
                            else:
                                ts.append(2 - 2 * d)
                        if ok:
                            W128[k, P * 16:P * 16 + 16] = w[cin, :, ts[0], ts[1], ts[2]]

    # pooled effective kernel: Weff[cin,c,a,b,g] = sum of w over tap product set
    Phi = np.zeros((3, 3), np.float32)
    Phi[0, 1] = Phi[0, 2] = 1
    Phi[1, :] = 1
    Phi[2, 0] = 1
    Weff = np.einsum("at,bu,gv,nctuv->ncabg", Phi, Phi, Phi, w).astype(np.float32)

    # WPT[(cin,bd,bh,bw), 16*p + c] : pass p=(od,oh,ow), tap s=b+2o (s=3 invalid)
    WPT = np.zeros((24, 128), np.float32)
    for p in range(8):
        od, oh, ow = p >> 2 & 1, p >> 1 & 1, p & 1
        for cin in range(3):
            for bd in range(2):
                for bh in range(2):
                    for bw in range(2):
                        sd, sh, sw = bd + 2 * od, bh + 2 * oh, bw + 2 * ow
                        if 3 in (sd, sh, sw):
                            continue
                        k = (bd * 4 + bh * 2 + bw) * 3 + cin
                        WPT[k, p * 16:p * 16 + 16] = Weff[cin, :, sd, sh, sw]

    # ones128[16*P + c, 32*s + c] = 1  (phase-sum + broadcast to per-sample rows)
    ONES = np.zeros((128, 128), np.float32)
    for P in range(8):
        for c in range(16):
            for s in range(4):
                ONES[P * 16 + c, 32 * s + c] = 1.0

    # region validity per phase row: MASK[row, r=fd*4+fh*2+fw]
    MASK = np.zeros((128, 8), np.float32)
    for P in range(8):
        ed, eh, ew = P >> 2 & 1, P >> 1 & 1, P & 1
        for r in range(8):
            fd, fh, fw = r >> 2 & 1, r >> 1 & 1, r & 1
            if (not fd or ed == 0) and (not fh or eh == 0) and (not fw or ew == 0):
                MASK[P * 16:P * 16 + 16, r] = 1.0
    # per-triple region weights: REGW[row, 2*t + parity] = MASK[row, region]
    REGW = np.zeros((128, 2 * _NSLOT), np.float32)
    t = 0
    for _s in range(SPC):
        for (ereg, oreg, *_rest) in _CH:
            REGW[:, 2 * t] = MASK[:, ereg]
            REGW[:, 2 * t + 1] = MASK[:, oreg]
            t += 1

    # ScalarE-chunk region weights (one col per ScalarE-assigned chunk)
    AREGW = np.zeros((128, _NACT), np.float32)
    a = 0
    for ci, (ereg, oreg, *_rest) in enumerate(_CH):
        for _s in range(SPC):
            if _act_assign(ci, _s):
                AREGW[:, a] = MASK[:, ereg]
                a += 1

    # gamma/beta tiled onto rows 32*s + c
    GB = np.zeros((128, 2), np.float32)
    for s in range(4):
        GB[32 * s:32 * s + 16, 0] = gamma
        GB[32 * s:32 * s + 16, 1] = beta
    return dict(w128=W128, wpt=WPT, ones=ONES, regw=REGW, aregw=AREGW, gb=GB)


# ---------------------------------------------------------------------------
# chunk schedule: (even_region, odd_region, d0, nd, h0, nh, w0, nw)
# All N = nd*nh*nw are even (fp32r matmul requirement). Columns stream with w
# innermost; chunks whose w-range is the pair {30,31} rely on bn_stats' even/
# odd-position split to separate valid (w=30) from face-w (w=31) statistics.
# ---------------------------------------------------------------------------
def _chunks():
    ch = []
    for md in range(31):                            # interior, w<30
        ch.append((0, 0, md, 1, 0, 16, 0, 30))
        ch.append((0, 0, md, 1, 16, 15, 0, 30))
    for m0, nm in ((0, 8), (8, 8), (16, 8), (24, 7)):   # w-pair, interior d/h
        ch.append((0, 1, m0, nm, 0, 31, 30, 2))
    ch.append((2, 2, 0, 16, 31, 1, 0, 30))          # face h
    ch.append((2, 2, 16, 15, 31, 1, 0, 30))
    ch.append((2, 3, 0, 31, 31, 1, 30, 2))          # face h, w-pair
    ch.append((4, 4, 31, 1, 0, 16, 0, 30))          # face d
    ch.append((4, 4, 31, 1, 16, 15, 0, 30))
    ch.append((4, 5, 31, 1, 0, 31, 30, 2))          # face d, w-pair
    ch.append((6, 6, 31, 1, 31, 1, 0, 30))          # edge dh
    ch.append((6, 7, 31, 1, 31, 1, 30, 2))          # edge dh, w-pair
    return ch


_CH = _chunks()
_NSLOT = SPC * len(_CH)                             # 296 ops; 2 triples each
# scan-engine split: single-region large chunks may have their (sum, sumsq)
# computed on ScalarE (2 activation ops w/ accum) instead of VectorE bn_stats.
# Assignment is per (chunk, sample) so each 4-sample matmul group feeds both
# engines concurrently (~1/3 to ScalarE).
def _act_assign(ci, s):
    er, orr, d0, nd, h0, nh, w0, nw = _CH[ci]
    return (er == orr) and (nd * nh * nw >= 100) and ((ci + s) % 3 == 0)


_NACT = sum(1 for _ci in range(len(_CH)) for _s in range(SPC) if _act_assign(_ci, _s))
# ---------------------------------------------------------------------------
# bass kernel builder
# ---------------------------------------------------------------------------
_BUILD_CACHE = {}


def build_nc(n_cores=NCORES):
    if n_cores in _BUILD_CACHE:
        return _BUILD_CACHE[n_cores]
    import concourse.bacc as bacc
    import concourse.tile as tile
    import concourse.mybir as mybir

    f32 = mybir.dt.float32
    ALU = mybir.AluOpType
    AFT = mybir.ActivationFunctionType
    CNT = float(n_cores * SPC * 63 ** 3)

    nc = bacc.Bacc(
        "TRN2",
        target_bir_lowering=False,
        debug=False,
        num_devices=n_cores,
    )
    f32r_ = mybir.dt.float32r
    xcat = nc.dram_tensor("xcat", [XCAT], f32r_, kind="ExternalInput")
    w128d = nc.dram_tensor("w128", [24, 128], f32r_, kind="ExternalInput")
    wptd = nc.dram_tensor("wpt", [24, 128], f32r_, kind="ExternalInput")
    onesd = nc.dram_tensor("ones", [128, 128], f32, kind="ExternalInput")
    maskd = nc.dram_tensor("regw", [128, 2 * _NSLOT], f32, kind="ExternalInput")
    aregwd = nc.dram_tensor("aregw", [128, _NACT], f32, kind="ExternalInput")
    gbd = nc.dram_tensor("gb", [128, 2], f32, kind="ExternalInput")
    outd = nc.dram_tensor("out", [SPC, 16, 3375], f32, kind="ExternalOutput")

    with tile.TileContext(nc) as tc:
        with (
            tc.tile_pool(name="big", bufs=1) as big,
            tc.tile_pool(name="cst", bufs=1) as cst,
            tc.tile_pool(name="sml", bufs=1) as sml,
            tc.tile_pool(name="dram", bufs=1, space="DRAM") as dram,
        ):
            f32r = mybir.dt.float32r
            V = big.tile([128, S], f32r)
            STATS = big.tile([128, _NSLOT * 6], f32)
            SCR1 = big.tile([128, 2 * _NSLOT], f32)
            SCR2 = big.tile([128, 2 * _NSLOT], f32)
            praw = big.tile([128, 3375], f32)
            staged = big.tile([128, 3375], f32)

            W128t = cst.tile([128, 128], f32r)
            WPTt = cst.tile([128, 128], f32r)
            ONESt = cst.tile([128, 128], f32)
            REGWt = cst.tile([128, 2 * _NSLOT], f32)
            AREGWt = cst.tile([128, _NACT], f32)
            ASUM = big.tile([128, _NACT], f32)
            ASQ = big.tile([128, _NACT], f32)
            SCRA = big.tile([128, 512], f32)
            SSA = sml.tile([128, 2], f32)
            GBt = cst.tile([128, 2], f32)

            SS = sml.tile([128, 2], f32)
            ssb = sml.tile([128, 2], f32)
            gss = sml.tile([128, 2], f32)
            meanT = sml.tile([128, 1], f32)
            ex2T = sml.tile([128, 1], f32)
            varT = sml.tile([128, 1], f32)
            sqT = sml.tile([128, 1], f32)
            invT = sml.tile([128, 1], f32)
            sclT = sml.tile([128, 1], f32)
            tmpT = sml.tile([128, 1], f32)
            biaT = sml.tile([128, 1], f32)

            nc.gpsimd.memset(praw[:, :], 0.0)
            nc.gpsimd.memset(STATS[:, :], 0.0)
            nc.gpsimd.memset(ASUM[:, :], 0.0)
            nc.gpsimd.memset(ASQ[:, :], 0.0)

            # ---- constants in ----
            for s in range(SPC):
                nc.sync.dma_start(W128t[32 * s:32 * s + 24, :], w128d[:, :])
                nc.sync.dma_start(WPTt[32 * s:32 * s + 24, :], wptd[:, :])
            nc.sync.dma_start(ONESt[:, :], onesd[:, :])
            nc.sync.dma_start(REGWt[:, :], maskd[:, :])
            nc.sync.dma_start(AREGWt[:, :], aregwd[:, :])
            nc.sync.dma_start(GBt[:, :], gbd[:, :])

            # ---- V build: rows (32s + delta*3 + cin) = xcat flat-shifted ----
            for s in range(SPC):
                for dd in range(2):
                    for dh in range(2):
                        for dw in range(2):
                            d = dd * 4 + dh * 2 + dw
                            off = s * 3 * S + dd * 1024 + dh * 32 + dw
                            src = xcat[off:off + 3 * S].rearrange("(c m) -> c m", m=S)
                            eng = nc.sync if d % 2 == 0 else nc.scalar
                            eng.dma_start(V[32 * s + 3 * d:32 * s + 3 * d + 3, :], src)

            # ---- main conv + bn_stats / scalar-accum scan ----
            # sample loop innermost: consecutive MMs hit different PE row
            # groups. conv-P (pooled) chunks are interleaved to keep the PE
            # dense; the last 3 run after the stats matmul to hide the
            # all-reduce latency.
            V4 = V.rearrange("p (d h w) -> p d h w", h=32, w=32)
            V4p = V.rearrange("p (d dp h hp w wp) -> p dp hp wp d h w",
                              d=16, dp=2, h=16, hp=2, w=16, wp=2)
            jchunks = [(0, 2), (2, 2), (4, 2), (6, 2), (8, 2), (10, 2), (12, 2), (13, 2)]
            psB_cm = tc.tile_pool(name="psB", bufs=1, space="PSUM")
            psB = psB_cm.__enter__()

            def convp_chunk(jd0, njd):
                NP = njd * 225
                pchunks = [psB.tile([128, 512], f32, name=f"pchunk{s}", tag=f"pchunk{s}")
                           for s in range(SPC)]
                for p in range(8):
                    od, oh, ow = p >> 2 & 1, p >> 1 & 1, p & 1
                    for s in range(SPC):
                        rhs = V4p[32 * s:32 * s + 24, 0, 0, 0,
                                  od + jd0:od + jd0 + njd,
                                  oh:oh + 15, ow:ow + 15]
                        nc.tensor.matmul(
                            pchunks[s][32 * s:32 * s + 16, 0:NP],
                            WPTt[32 * s:32 * s + 24, 16 * p:16 * p + 16].bitcast(f32),
                            rhs.bitcast(f32),
                            start=(p == 0), stop=(p == 7),
                            tile_position=(32 * s, 32 * s),
                        )
                for s in range(SPC):
                    nc.scalar.copy(
                        praw[32 * s:32 * s + 16, 225 * jd0:225 * jd0 + NP],
                        pchunks[s][32 * s:32 * s + 16, 0:NP],
                    )

            nch = len(_CH)
            act_slot = 0
            with tc.tile_pool(name="psA", bufs=4, space="PSUM") as psA:
                for ci, (ereg, oreg, d0, nd, h0, nh, w0, nw) in enumerate(_CH):
                    N = nd * nh * nw
                    for s in range(SPC):
                        chunk = psA.tile([128, 512], f32, tag="chunk")
                        rhs = V4[32 * s:32 * s + 24, d0:d0 + nd, h0:h0 + nh, w0:w0 + nw]
                        nc.tensor.matmul(
                            chunk[:, 0:N],
                            W128t[32 * s:32 * s + 24, :],
                            rhs,
                            start=True, stop=True,
                            tile_position=(32 * s, 0),
                        )
                        if _act_assign(ci, s):
                            a = act_slot
                            act_slot += 1
                            nc.scalar.activation(SCRA[:, 0:N], chunk[:, 0:N],
                                                 AFT.Square,
                                                 accum_out=ASQ[:, a:a + 1])
                            nc.scalar.activation(SCRA[:, 0:N], chunk[:, 0:N],
                                                 AFT.Identity,
                                                 accum_out=ASUM[:, a:a + 1])
                        else:
                            t = s * nch + ci
                            nc.vector.bn_stats(STATS[:, 6 * t:6 * t + 6], chunk[:, 0:N])
                    if ci % 10 == 9 and ci // 10 < 5:
                        convp_chunk(*jchunks[ci // 10])

            # ---- stats finalize (region weights applied per bn_stats triple) ----
            st3 = STATS.rearrange("p (n t) -> p n t", t=3)
            counts = st3[:, :, 0]
            means = st3[:, :, 1]
            cvs = st3[:, :, 2]
            nc.vector.tensor_tensor(out=SCR1[:, :], in0=counts, in1=means, op=ALU.mult)
            nc.vector.tensor_tensor(out=SCR2[:, :], in0=SCR1[:, :], in1=means, op=ALU.mult)
            nc.vector.tensor_tensor(out=SCR2[:, :], in0=SCR2[:, :], in1=cvs, op=ALU.add)
            nc.vector.tensor_tensor(out=SCR2[:, :], in0=SCR2[:, :], in1=REGWt[:, :], op=ALU.mult)
            nc.vector.reduce_sum(SS[:, 1:2], SCR2[:, :], axis=mybir.AxisListType.X)
            nc.vector.tensor_tensor(out=SCR1[:, :], in0=SCR1[:, :], in1=REGWt[:, :], op=ALU.mult)
            nc.vector.reduce_sum(SS[:, 0:1], SCR1[:, :], axis=mybir.AxisListType.X)
            # merge ScalarE-chunk sums
            nc.vector.tensor_tensor(out=SCR1[:, 0:_NACT], in0=ASUM[:, :], in1=AREGWt[:, :], op=ALU.mult)
            nc.vector.reduce_sum(SSA[:, 0:1], SCR1[:, 0:_NACT], axis=mybir.AxisListType.X)
            nc.vector.tensor_tensor(out=SCR1[:, 0:_NACT], in0=ASQ[:, :], in1=AREGWt[:, :], op=ALU.mult)
            nc.vector.reduce_sum(SSA[:, 1:2], SCR1[:, 0:_NACT], axis=mybir.AxisListType.X)
            nc.vector.tensor_tensor(out=SS[:, :], in0=SS[:, :], in1=SSA[:, :], op=ALU.add)

            # phase-sum + broadcast to per-sample channel rows: [128,2] psum
            pss = psB.tile([128, 2], f32, tag="pchunk0")
            nc.tensor.matmul(pss[:, :], ONESt[:, :], SS[:, :], start=True, stop=True)
            nc.vector.tensor_copy(ssb[:, :], pss[:, :])

            # ---- sync-BN all-reduce across cores ----
            import os
            if n_cores > 1 and not os.environ.get("KERNEL_NO_CC"):
                cin_b = dram.tile([128, 2], f32)
                cout_b = dram.tile([128, 2], f32)
                nc.gpsimd.dma_start(cin_b[:, :], ssb[:, :])
                nc.gpsimd.collective_compute(
                    "AllReduce",
                    ALU.add,
                    replica_groups=[list(range(n_cores))],
                    ins=[cin_b.opt()],
                    outs=[cout_b.opt()],
                )
                nc.gpsimd.dma_start(gss[:, :], cout_b[:, :])
            else:
                nc.vector.tensor_copy(gss[:, :], ssb[:, :])

            # last pooled-conv chunks run here, overlapping the all-reduce
            for jd0, njd in jchunks[5:]:
                convp_chunk(jd0, njd)

            # ---- finalize scalars ----
            nc.vector.tensor_scalar_mul(meanT[:, :], gss[:, 0:1], 1.0 / CNT)
            nc.vector.tensor_scalar_mul(ex2T[:, :], gss[:, 1:2], 1.0 / CNT)
            nc.vector.tensor_tensor(out=varT[:, :], in0=meanT[:, :], in1=meanT[:, :], op=ALU.mult)
            nc.vector.tensor_tensor(out=varT[:, :], in0=ex2T[:, :], in1=varT[:, :], op=ALU.subtract)
            nc.vector.tensor_scalar_add(varT[:, :], varT[:, :], 1e-5)
            nc.scalar.activation(sqT[:, :], varT[:, :], AFT.Sqrt)
            nc.vector.reciprocal(invT[:, :], sqT[:, :])
            # scale = inv * gamma / 64 ; bias = beta - mean * inv * gamma
            nc.vector.tensor_tensor(out=sclT[:, :], in0=invT[:, :], in1=GBt[:, 0:1], op=ALU.mult)
            nc.vector.tensor_tensor(out=tmpT[:, :], in0=meanT[:, :], in1=sclT[:, :], op=ALU.mult)
            nc.vector.tensor_tensor(out=biaT[:, :], in0=GBt[:, 1:2], in1=tmpT[:, :], op=ALU.subtract)
            nc.vector.tensor_scalar_mul(sclT[:, :], sclT[:, :], 1.0 / 64.0)

            # ---- normalize + out ----
            nc.scalar.activation(staged[:, :], praw[:, :], AFT.Identity,
                                 bias=biaT[:, 0:1], scale=sclT[:, 0:1])
            for s in range(SPC):
                nc.sync.dma_start(outd[s], staged[32 * s:32 * s + 16, :])
            psB_cm.__exit__(None, None, None)

    nc.compile()
    _BUILD_CACHE[n_cores] = nc
    return nc


# ---------------------------------------------------------------------------
# host entry point
# ---------------------------------------------------------------------------
def make_in_maps(x, weight, gamma, beta, n_cores=NCORES):
    x = np.ascontiguousarray(np.asarray(x, np.float32))
    consts = _host_consts(weight, np.asarray(gamma, np.float32), np.asarray(beta, np.float32))
    in_maps = []
    for core in range(n_cores):
        xs = x[core * SPC:(core + 1) * SPC]
        xc = np.zeros(XCAT, np.float32)
        xc[:SPC * 3 * S] = xs.reshape(-1)
        in_maps.append({
            "xcat": xc,
            "w128": consts["w128"],
            "wpt": consts["wpt"],
            "ones": consts["ones"],
            "regw": consts["regw"],
            "aregw": consts["aregw"],
            "gb": consts["gb"],
        })
    return in_maps


def kernel(x, weight, gamma, beta):
    import sys
    if "/opt/trn_rl_repo" not in sys.path:
        sys.path.insert(0, "/opt/trn_rl_repo")
    from concourse.bass_utils import run_bass_kernel_spmd

    nc = build_nc(NCORES)
    in_maps = make_in_maps(x, weight, gamma, beta, NCORES)
    res = run_bass_kernel_spmd(nc, in_maps, core_ids=list(range(NCORES)))
    outs = [r["out"].reshape(SPC, 16, 15, 15, 15) for r in res.results]
    return np.concatenate(outs, axis=0)


if __name__ == "__main__":
    import sys
    sys.path.insert(0, "/opt/trn_rl_repo")
    sys.path.insert(0, "/root/problem")
    import reference as ref
    inputs = {k: np.asarray(v) for k, v in ref.setup_inputs().items()}
    out = kernel(**inputs)
    print("out shape", out.shape)


# revision 32
# speedup vs baseline: 1.3351x; 1.3351x over previous
"""Trainium2 Bass kernel for ConvTranspose3d(3->16,k3,s2,p1) + BatchNorm3d(train) + 2x AvgPool3d(2).

Algorithm (per core, batch-sharded 4 samples/core over 8 cores):
  - ConvT decomposes into 8 "phases" (output parity per spatial dim). With the input
    replicated 8x as flat-shifted rows V[(cin,dd,dh,dw)] = x[cin].flat[shift:], one
    matmul per output-base-position chunk computes all 8 phases x 16 channels at once
    (lhsT = phase-weight matrix [24,128], rhs = V rows [24,N], out = [128,N] PSUM).
  - BN statistics (sum, sumsq per channel) come from bn_stats over each PSUM chunk,
    region-split (interior/faces/edges/corner) so invalid phase outputs are excluded,
    then a cross-core AllReduce (sync-BN).
  - The two AvgPools collapse into a 4x4x4 block-sum of conv output positions 0..59,
    which equals a stride-2 3x3x3 conv of x with a host-precomputed effective kernel
    (BN is per-channel affine, so pooling commutes with it). Computed by a second,
    tiny 8-pass matmul stack; normalization is applied as a single fused affine.
"""

import numpy as np

S = 32768          # 32*32*32 flat spatial
SPC = 4            # samples per core
NCORES = 8
PAD = 2048
XCAT = SPC * 3 * S + PAD


# ---------------------------------------------------------------------------
# host-side constant construction (weight transforms etc.)
# ---------------------------------------------------------------------------
def _host_consts(weight, gamma, beta):
    w = np.asarray(weight, np.float32)            # (3,16,3,3,3)

    # W128[(dd,dh,dw,cin), 16*P + c], P = 4*ed+2*eh+ew   (delta-major rows)
    W128 = np.zeros((24, 128), np.float32)
    for cin in range(3):
        for dd in range(2):
            for dh in range(2):
                for dw in range(2):
                    k = cin * 8 + dd * 4 + dh * 2 + dw
                    for P in range(8):
                        ed, eh, ew = P >> 2 & 1, P >> 1 & 1, P & 1
                        ok, ts = True, []
                        for e, d in ((ed, dd), (eh, dh), (ew, dw)):
                            if e == 0:
                                if d != 0:
                                    ok = False
                                    break
                                ts.append(1)
                            else:
                                ts.append(2 - 2 * d)
                        if ok:
                            W128[k, P * 16:P * 16 + 16] = w[cin, :, ts[0], ts[1], ts[2]]

    # pooled effective kernel: Weff[cin,c,a,b,g] = sum of w over tap product set
    Phi = np.zeros((3, 3), np.float32)
    Phi[0, 1] = Phi[0, 2] = 1
    Phi[1, :] = 1
    Phi[2, 0] = 1
    Weff = np.einsum("at,bu,gv,nctuv->ncabg", Phi, Phi, Phi, w).astype(np.float32)

    # WPT[(cin,bd,bh,bw), 16*p + c] : pass p=(od,oh,ow), tap s=b+2o (s=3 invalid)
    WPT = np.zeros((24, 128), np.float32)
    for p in range(8):
        od, oh, ow = p >> 2 & 1, p >> 1 & 1, p & 1
        for cin in range(3):
            for bd in range(2):
                for bh in range(2):
                    for bw in range(2):
                        sd, sh, sw = bd + 2 * od, bh + 2 * oh, bw + 2 * ow
                        if 3 in (sd, sh, sw):
                            continue
                        k = cin * 8 + bd * 4 + bh * 2 + bw
                        WPT[k, p * 16:p * 16 + 16] = Weff[cin, :, sd, sh, sw]

    # ones128[16*P + c, 32*s + c] = 1  (phase-sum + broadcast to per-sample rows)
    ONES = np.zeros((128, 128), np.float32)
    for P in range(8):
        for c in range(16):
            for s in range(4):
                ONES[P * 16 + c, 32 * s + c] = 1.0

    # region validity per phase row: MASK[row, r=fd*4+fh*2+fw]
    MASK = np.zeros((128, 8), np.float32)
    for P in range(8):
        ed, eh, ew = P >> 2 & 1, P >> 1 & 1, P & 1
        for r in range(8):
            fd, fh, fw = r >> 2 & 1, r >> 1 & 1, r & 1
            if (not fd or ed == 0) and (not fh or eh == 0) and (not fw or ew == 0):
                MASK[P * 16:P * 16 + 16, r] = 1.0
    # per-triple region weights: REGW[row, 2*t + parity] = MASK[row, region]
    REGW = np.zeros((128, 2 * _NSLOT), np.float32)
    t = 0
    for _s in range(SPC):
        for (ereg, oreg, *_rest) in _CH:
            REGW[:, 2 * t] = MASK[:, ereg]
            REGW[:, 2 * t + 1] = MASK[:, oreg]
            t += 1

    # ScalarE-chunk region weights (one col per ScalarE-assigned chunk)
    AREGW = np.zeros((128, _NACT), np.float32)
    a = 0
    for ci, (ereg, oreg, *_rest) in enumerate(_CH):
        for _s in range(SPC):
            if _act_assign(ci, _s):
                AREGW[:, a] = MASK[:, ereg]
                a += 1

    # gamma/beta tiled onto rows 32*s + c
    GB = np.zeros((128, 2), np.float32)
    for s in range(4):
        GB[32 * s:32 * s + 16, 0] = gamma
        GB[32 * s:32 * s + 16, 1] = beta
    return dict(w128=W128, wpt=WPT, ones=ONES, regw=REGW, aregw=AREGW, gb=GB)


# ---------------------------------------------------------------------------
# chunk schedule: (even_region, odd_region, d0, nd, h0, nh, w0, nw)
# All N = nd*nh*nw are even (fp32r matmul requirement). Columns stream with w
# innermost; chunks whose w-range is the pair {30,31} rely on bn_stats' even/
# odd-position split to separate valid (w=30) from face-w (w=31) statistics.
# ---------------------------------------------------------------------------
def _chunks():
    ch = []
    for md in range(31):                            # interior, w<30
        ch.append((0, 0, md, 1, 0, 16, 0, 30))
        ch.append((0, 0, md, 1, 16, 15, 0, 30))
    for m0, nm in ((0, 8), (8, 8), (16, 8), (24, 7)):   # w-pair, interior d/h
        ch.append((0, 1, m0, nm, 0, 31, 30, 2))
    ch.append((2, 2, 0, 16, 31, 1, 0, 30))          # face h
    ch.append((2, 2, 16, 15, 31, 1, 0, 30))
    ch.append((2, 3, 0, 31, 31, 1, 30, 2))          # face h, w-pair
    ch.append((4, 4, 31, 1, 0, 16, 0, 30))          # face d
    ch.append((4, 4, 31, 1, 16, 15, 0, 30))
    ch.append((4, 5, 31, 1, 0, 31, 30, 2))          # face d, w-pair
    ch.append((6, 6, 31, 1, 31, 1, 0, 30))          # edge dh
    ch.append((6, 7, 31, 1, 31, 1, 30, 2))          # edge dh, w-pair
    return ch


_CH = _chunks()
_NSLOT = SPC * len(_CH)                             # 296 ops; 2 triples each
# scan-engine split: single-region large chunks may have their (sum, sumsq)
# computed on ScalarE (2 activation ops w/ accum) instead of VectorE bn_stats.
# Assignment is per (chunk, sample) so each 4-sample matmul group feeds both
# engines concurrently (~1/3 to ScalarE).
def _act_assign(ci, s):
    er, orr, d0, nd, h0, nh, w0, nw = _CH[ci]
    return (er == orr) and (nd * nh * nw >= 100) and ((ci + s) % 5 in (0, 2))


_NACT = sum(1 for _ci in range(len(_CH)) for _s in range(SPC) if _act_assign(_ci, _s))
# ---------------------------------------------------------------------------
# bass kernel builder
# ---------------------------------------------------------------------------
_BUILD_CACHE = {}


def build_nc(n_cores=NCORES):
    if n_cores in _BUILD_CACHE:
        return _BUILD_CACHE[n_cores]
    import concourse.bacc as bacc
    import concourse.tile as tile
    import concourse.mybir as mybir

    f32 = mybir.dt.float32
    ALU = mybir.AluOpType
    AFT = mybir.ActivationFunctionType
    CNT = float(n_cores * SPC * 63 ** 3)

    nc = bacc.Bacc(
        "TRN2",
        target_bir_lowering=False,
        debug=False,
        num_devices=n_cores,
    )
    f32r_ = mybir.dt.float32r
    xcat = nc.dram_tensor("xcat", [XCAT], f32r_, kind="ExternalInput")
    w128d = nc.dram_tensor("w128", [24, 128], f32r_, kind="ExternalInput")
    wptd = nc.dram_tensor("wpt", [24, 128], f32r_, kind="ExternalInput")
    onesd = nc.dram_tensor("ones", [128, 128], f32, kind="ExternalInput")
    maskd = nc.dram_tensor("regw", [128, 2 * _NSLOT], f32, kind="ExternalInput")
    aregwd = nc.dram_tensor("aregw", [128, _NACT], f32, kind="ExternalInput")
    gbd = nc.dram_tensor("gb", [128, 2], f32, kind="ExternalInput")
    outd = nc.dram_tensor("out", [SPC, 16, 3375], f32, kind="ExternalOutput")

    with tile.TileContext(nc) as tc:
        with (
            tc.tile_pool(name="big", bufs=1) as big,
            tc.tile_pool(name="cst", bufs=1) as cst,
            tc.tile_pool(name="sml", bufs=1) as sml,
            tc.tile_pool(name="dram", bufs=1, space="DRAM") as dram,
        ):
            f32r = mybir.dt.float32r
            V = big.tile([128, S], f32r)
            STATS = big.tile([128, _NSLOT * 6], f32)
            SCR1 = big.tile([128, 2 * _NSLOT], f32)
            SCR2 = big.tile([128, 2 * _NSLOT], f32)
            praw = big.tile([128, 3375], f32)
            staged = big.tile([128, 3375], f32)

            W128t = cst.tile([128, 128], f32r)
            WPTt = cst.tile([128, 128], f32r)
            ONESt = cst.tile([128, 128], f32)
            REGWt = cst.tile([128, 2 * _NSLOT], f32)
            AREGWt = cst.tile([128, _NACT], f32)
            ASUM = big.tile([128, _NACT], f32)
            ASQ = big.tile([128, _NACT], f32)
            SCRA = big.tile([128, 512], f32)
            SSA = sml.tile([128, 2], f32)
            GBt = cst.tile([128, 2], f32)

            SS = sml.tile([128, 2], f32)
            ssb = sml.tile([128, 2], f32)
            gss = sml.tile([128, 2], f32)
            meanT = sml.tile([128, 1], f32)
            ex2T = sml.tile([128, 1], f32)
            varT = sml.tile([128, 1], f32)
            sqT = sml.tile([128, 1], f32)
            invT = sml.tile([128, 1], f32)
            sclT = sml.tile([128, 1], f32)
            tmpT = sml.tile([128, 1], f32)
            biaT = sml.tile([128, 1], f32)

            nc.gpsimd.memset(praw[:, :], 0.0)
            nc.gpsimd.memset(STATS[:, :], 0.0)
            nc.gpsimd.memset(ASUM[:, :], 0.0)
            nc.gpsimd.memset(ASQ[:, :], 0.0)

            # ---- constants in ----
            for s in range(SPC):
                nc.gpsimd.dma_start(W128t[32 * s:32 * s + 24, :], w128d[:, :])
                nc.gpsimd.dma_start(WPTt[32 * s:32 * s + 24, :], wptd[:, :])
            nc.gpsimd.dma_start(ONESt[:, :], onesd[:, :])
            nc.gpsimd.dma_start(REGWt[:, :], maskd[:, :])
            nc.gpsimd.dma_start(AREGWt[:, :], aregwd[:, :])
            nc.gpsimd.dma_start(GBt[:, :], gbd[:, :])

            # ---- V build: rows (32s + cin*8 + delta) = xcat flat-shifted ----
            # rows cin-major so each DMA's 3 partitions hit 3 different SBUF
            # ports; spread across both HWDGE rings (sync + scalar)
            Vv = V.rearrange("(s c k) m -> s c k m", s=4, c=4, k=8)
            for s in range(SPC):
                for dd in range(2):
                    for dh in range(2):
                        for dw in range(2):
                            d = dd * 4 + dh * 2 + dw
                            off = s * 3 * S + dd * 1024 + dh * 32 + dw
                            src = xcat[off:off + 3 * S].rearrange("(c m) -> c m", m=S)
                            eng = nc.sync if d % 2 == 0 else nc.scalar
                            eng.dma_start(Vv[s, 0:3, d, :], src)

            # ---- main conv + bn_stats / scalar-accum scan ----
            # sample loop innermost: consecutive MMs hit different PE row
            # groups. conv-P (pooled) chunks are interleaved to keep the PE
            # dense; the last 3 run after the stats matmul to hide the
            # all-reduce latency.
            V4 = V.rearrange("p (d h w) -> p d h w", h=32, w=32)
            V4p = V.rearrange("p (d dp h hp w wp) -> p dp hp wp d h w",
                              d=16, dp=2, h=16, hp=2, w=16, wp=2)
            jchunks = [(0, 2), (2, 2), (4, 2), (6, 2), (8, 2), (10, 2), (12, 2), (13, 2)]
            psB = None

            def convp_chunk(jd0, njd):
                NP = njd * 225
                pchunks = [psB.tile([128, 512], f32, name=f"pchunk{s}", tag=f"pchunk{s}")
                           for s in range(SPC)]
                for p in range(8):
                    od, oh, ow = p >> 2 & 1, p >> 1 & 1, p & 1
                    for s in range(SPC):
                        rhs = V4p[32 * s:32 * s + 24, 0, 0, 0,
                                  od + jd0:od + jd0 + njd,
                                  oh:oh + 15, ow:ow + 15]
                        nc.tensor.matmul(
                            pchunks[s][32 * s:32 * s + 16, 0:NP],
                            WPTt[32 * s:32 * s + 24, 16 * p:16 * p + 16].bitcast(f32),
                            rhs.bitcast(f32),
                            start=(p == 0), stop=(p == 7),
                            tile_position=(32 * s, 32 * s),
                        )
                for s in range(SPC):
                    dst = praw[32 * s:32 * s + 16, 225 * jd0:225 * jd0 + NP]
                    srcp = pchunks[s][32 * s:32 * s + 16, 0:NP]
                    if s % 2 == 0:
                        nc.vector.tensor_copy(dst, srcp)
                    else:
                        nc.scalar.copy(dst, srcp)

            nch = len(_CH)
            act_slot = 0
            with tc.tile_pool(name="psA", bufs=6, space="PSUM") as psA:
                for ci, (ereg, oreg, d0, nd, h0, nh, w0, nw) in enumerate(_CH):
                    N = nd * nh * nw
                    for s in range(SPC):
                        chunk = psA.tile([128, 512], f32, tag="chunk")
                        rhs = V4[32 * s:32 * s + 24, d0:d0 + nd, h0:h0 + nh, w0:w0 + nw]
                        nc.tensor.matmul(
                            chunk[:, 0:N],
                            W128t[32 * s:32 * s + 24, :],
                            rhs,
                            start=True, stop=True,
                            tile_position=(32 * s, 0),
                        )
                        if _act_assign(ci, s):
                            a = act_slot
                            act_slot += 1
                            nc.scalar.activation(SCRA[:, 0:N], chunk[:, 0:N],
                                                 AFT.Square,
                                                 accum_out=ASQ[:, a:a + 1])
                            nc.scalar.activation(SCRA[:, 0:N], chunk[:, 0:N],
                                                 AFT.Identity,
                                                 accum_out=ASUM[:, a:a + 1])
                        else:
                            t = s * nch + ci
                            nc.vector.bn_stats(STATS[:, 6 * t:6 * t + 6], chunk[:, 0:N])

            # ---- stats finalize (region weights applied per bn_stats triple) ----
            st3 = STATS.rearrange("p (n t) -> p n t", t=3)
            counts = st3[:, :, 0]
            means = st3[:, :, 1]
            cvs = st3[:, :, 2]
            nc.vector.tensor_tensor(out=SCR1[:, :], in0=counts, in1=means, op=ALU.mult)
            nc.vector.tensor_tensor(out=SCR2[:, :], in0=SCR1[:, :], in1=means, op=ALU.mult)
            nc.vector.tensor_tensor(out=SCR2[:, :], in0=SCR2[:, :], in1=cvs, op=ALU.add)
            nc.vector.tensor_tensor(out=SCR2[:, :], in0=SCR2[:, :], in1=REGWt[:, :], op=ALU.mult)
            nc.vector.reduce_sum(SS[:, 1:2], SCR2[:, :], axis=mybir.AxisListType.X)
            nc.vector.tensor_tensor(out=SCR1[:, :], in0=SCR1[:, :], in1=REGWt[:, :], op=ALU.mult)
            nc.vector.reduce_sum(SS[:, 0:1], SCR1[:, :], axis=mybir.AxisListType.X)
            # merge ScalarE-chunk sums
            nc.vector.tensor_tensor(out=SCR1[:, 0:_NACT], in0=ASUM[:, :], in1=AREGWt[:, :], op=ALU.mult)
            nc.vector.reduce_sum(SSA[:, 0:1], SCR1[:, 0:_NACT], axis=mybir.AxisListType.X)
            nc.vector.tensor_tensor(out=SCR1[:, 0:_NACT], in0=ASQ[:, :], in1=AREGWt[:, :], op=ALU.mult)
            nc.vector.reduce_sum(SSA[:, 1:2], SCR1[:, 0:_NACT], axis=mybir.AxisListType.X)
            nc.vector.tensor_tensor(out=SS[:, :], in0=SS[:, :], in1=SSA[:, :], op=ALU.add)

            # phase-sum + broadcast to per-sample channel rows: [128,2] psum
            psB_cm = tc.tile_pool(name="psB", bufs=1, space="PSUM")
            psB = psB_cm.__enter__()
            pss = psB.tile([128, 2], f32, tag="pchunk0")
            nc.tensor.matmul(pss[:, :], ONESt[:, :], SS[:, :], start=True, stop=True)
            nc.vector.tensor_copy(ssb[:, :], pss[:, :])

            # ---- sync-BN all-reduce across cores ----
            import os
            if n_cores > 1 and not os.environ.get("KERNEL_NO_CC"):
                cin_b = dram.tile([128, 2], f32)
                cout_b = dram.tile([128, 2], f32)
                nc.gpsimd.dma_start(cin_b[:, :], ssb[:, :])
                nc.gpsimd.collective_compute(
                    "AllReduce",
                    ALU.add,
                    replica_groups=[list(range(n_cores))],
                    ins=[cin_b.opt()],
                    outs=[cout_b.opt()],
                )
                nc.gpsimd.dma_start(gss[:, :], cout_b[:, :])
            else:
                nc.vector.tensor_copy(gss[:, :], ssb[:, :])

            # pooled conv runs after the stats matmul: PE covers the
            # all-reduce latency with useful work
            for jd0, njd in jchunks:
                convp_chunk(jd0, njd)

            # ---- finalize scalars ----
            nc.vector.tensor_scalar_mul(meanT[:, :], gss[:, 0:1], 1.0 / CNT)
            nc.vector.tensor_scalar_mul(ex2T[:, :], gss[:, 1:2], 1.0 / CNT)
            nc.vector.tensor_tensor(out=varT[:, :], in0=meanT[:, :], in1=meanT[:, :], op=ALU.mult)
            nc.vector.tensor_tensor(out=varT[:, :], in0=ex2T[:, :], in1=varT[:, :], op=ALU.subtract)
            nc.vector.tensor_scalar_add(varT[:, :], varT[:, :], 1e-5)
            nc.scalar.activation(sqT[:, :], varT[:, :], AFT.Sqrt)
            nc.vector.reciprocal(invT[:, :], sqT[:, :])
            # scale = inv * gamma / 64 ; bias = beta - mean * inv * gamma
            nc.vector.tensor_tensor(out=sclT[:, :], in0=invT[:, :], in1=GBt[:, 0:1], op=ALU.mult)
            nc.vector.tensor_tensor(out=tmpT[:, :], in0=meanT[:, :], in1=sclT[:, :], op=ALU.mult)
            nc.vector.tensor_tensor(out=biaT[:, :], in0=GBt[:, 1:2], in1=tmpT[:, :], op=ALU.subtract)
            nc.vector.tensor_scalar_mul(sclT[:, :], sclT[:, :], 1.0 / 64.0)

            # ---- normalize + out ----
            nc.scalar.activation(staged[:, :], praw[:, :], AFT.Identity,
                                 bias=biaT[:, 0:1], scale=sclT[:, 0:1])
            for s in range(SPC):
                nc.sync.dma_start(outd[s], staged[32 * s:32 * s + 16, :])
            psB_cm.__exit__(None, None, None)

    nc.compile()
    _BUILD_CACHE[n_cores] = nc
    return nc


# ---------------------------------------------------------------------------
# host entry point
# ---------------------------------------------------------------------------
def make_in_maps(x, weight, gamma, beta, n_cores=NCORES):
    x = np.ascontiguousarray(np.asarray(x, np.float32))
    consts = _host_consts(weight, np.asarray(gamma, np.float32), np.asarray(beta, np.float32))
    in_maps = []
    for core in range(n_cores):
        xs = x[core * SPC:(core + 1) * SPC]
        xc = np.zeros(XCAT, np.float32)
        xc[:SPC * 3 * S] = xs.reshape(-1)
        in_maps.append({
            "xcat": xc,
            "w128": consts["w128"],
            "wpt": consts["wpt"],
            "ones": consts["ones"],
            "regw": consts["regw"],
            "aregw": consts["aregw"],
            "gb": consts["gb"],
        })
    return in_maps


def kernel(x, weight, gamma, beta):
    import sys
    if "/opt/trn_rl_repo" not in sys.path:
        sys.path.insert(0, "/opt/trn_rl_repo")
    from concourse.bass_utils import run_bass_kernel_spmd

    nc = build_nc(NCORES)
    in_maps = make_in_maps(x, weight, gamma, beta, NCORES)
    res = run_bass_kernel_spmd(nc, in_maps, core_ids=list(range(NCORES)))
    outs = [r["out"].reshape(SPC, 16, 15, 15, 15) for r in res.results]
    return np.concatenate(outs, axis=0)


if __name__ == "__main__":
    import sys
    sys.path.insert(0, "/opt/trn_rl_repo")
    sys.path.insert(0, "/root/problem")
    import reference as ref
    inputs = {k: np.asarray(v) for k, v in ref.setup_inputs().items()}
    out = kernel(**inputs)
    print("out shape", out.shape)
